# revision 14
# baseline (speedup 1.0000x reference)
"""Trainium2 Bass kernel for nn_EnhanceDiversityFeatureExtracition.

Computes  loss = mean((x-y)^2) + ALPHA * diversity_reg(conv_w)
where diversity_reg builds a 64x64 Gram matrix of the F=64 slices
conv_w[:, :, i, :] (each flattened to a 786432-vector), normalizes it to
cosine similarities, and sums the entries with tau < sim <= 1 off the
diagonal.

Distribution (8 NeuronCores, SPMD):
  - x_batch / y_batch sharded on batch dim: 256 rows per core.
  - conv_w viewed as A = conv_w.reshape(262144, 192)  (row m = (o,c),
    col g = f*3+k).  gram[i,j] = sum_k C[3i+k, 3j+k] where C = A^T A;
    A is sharded along the reduction axis: 32768 rows per core.  C is
    symmetric: each core computes C[0:128, 0:192] (cps1) and
    C[128:192, 128:192] (cps2) only; the host mirrors the rest.

Precision/bandwidth tradeoffs (this is a memory-bound kernel; the
rel-err gate is 2e-2 and the similarity threshold margin is ~0.2):
  - A is pre-scaled by 32 on the host and cast to fp8 e4m3 (the scale
    cancels in sim = gram/(n_i n_j); x32 centers N(0, 0.05) data in
    e4m3's normal range).  Per-element rounding is ~4%, but sims are
    dots of 786k-element unit vectors, so the induced sim error is
    ~1e-4 against the 0.2 threshold margin.  HBM traffic for A drops
    4x vs f32 (6MB/core), and fp8 matmuls stream 2 cols/cycle.
  - x/y are pre-scaled by 4 and cast to fp8 e4m3 on the host (the
    host divides the returned SSE by 16).  Input quantization biases
    mean((x-y)^2) by ~2*(0.036)^2 ~ 2.6e-3 relative -- an 8x margin
    against the 2e-2 gate.  1MB/core each.
  Total HBM per core: 8MB vs 32MB for the all-f32 variant.

On-core dataflow (~26us of DMA at the ~390GB/s/core ceiling):
  - A streams in 16 chunks of 2048 rows as [128 x 3072B] fp8
    (3KB per-partition descriptors), chunks alternating between the
    two HWDGE rings (sync/act) so issue latency and per-queue
    completion waits overlap.  No staging casts: matmuls consume fp8
    directly, so the only A dependency is DMA -> MM.
  - Tiles are processed in pairs with DoubleRow perf mode (fp8-only,
    contraction 256, 0.5 cycles/moving-row): per pair,
    cps1 += sum_i pair_i[:, 0:128]^T @ pair_i (moving 192) and
    cps2 += sum_i pair_i[:, 128:192]^T @ pair_i[:, 128:192].  This
    halves both matmul cycles and instruction count vs single-row --
    without it the PE (1 cycle/row even for fp8) drains ~10us past
    the last DMA.  Each chunk contributes a contiguous run of 8
    same-PSUM matmuls -- interleaving accumulation groups
    per-instruction breaks MM pipelining.
  - MSE: 4 [128 x 2048] fp8 pieces per operand (2KB per-partition
    descriptors); piece p loads x at chunk 4p and y at chunk 4p+1 on
    the ring opposite that chunk's A stream.  DVE computes d = x-y
    into bf16 as two 1024-col halves; ACT squares each half with a
    per-partition accumulate into its own acc column.  Each Square is
    emitted 2-3 chunks after its y load, so by the time the act-ring
    sequencer reaches it the subtract has long finished -- a Square
    whose dependency is pending stalls all later A-chunk issues on
    that ring (a full-size Square cost ~3us of stream here).
    (tensor_tensor_reduce on DVE would fuse sub+square+reduce but
    fails to compile/run on hardware in this stack.)
  - The final A chunk streams as two 1024-row halves with their own
    matmul groups, halving the post-stream PE drain.
  - The SSE partials ride as 8 extra columns of the c1 writeback: a
    separate tiny-descriptor DMA mid-stream used to stall the A
    stream ~6us via the shared per-queue completion counters.
"""

import numpy as np
import ml_dtypes

import concourse.bass as bass
import concourse.mybir as mybir
from concourse import bacc, tile
from concourse.bass_utils import run_bass_kernel_spmd

N_CORES = 8
B, D = 2048, 4096            # x_batch / y_batch
M, G = 262144, 192           # conv_w as (M, G); G = F*KW
F, KW = 64, 3
ROWS = B // N_CORES          # 256 batch rows per core
MC = M // N_CORES            # 32768 reduction rows per core
NCHK = 16                    # A chunks per core (2048 rows each)
TPC = MC // NCHK // 128      # 16 tiles of 128 rows per chunk
NCH = 4                      # MSE pieces per core
CHW = (ROWS * D) // (128 * NCH)  # 2048 elems per partition per piece
CHH = CHW // 2               # sub/Square half width
NACC = 2 * NCH               # one acc column per Square half
ASCALE = 32.0                # fp8 pre-scale; cancels in sim
XSCALE = 4.0                 # x/y fp8 pre-scale; host divides SSE by 16

ALPHA = 0.0005
TAU = 0.2

_prog = None


def _build() -> bass.Bass:
    nc = bacc.Bacc(None, target_bir_lowering=False)
    f32 = mybir.dt.float32
    bf16 = mybir.dt.bfloat16
    fp8 = mybir.dt.float8e4

    xs = nc.dram_tensor("xs", [ROWS, D], fp8, kind="ExternalInput")
    ys = nc.dram_tensor("ys", [ROWS, D], fp8, kind="ExternalInput")
    aw = nc.dram_tensor("aw", [MC, G], fp8, kind="ExternalInput")
    c1_part = nc.dram_tensor("c1_part", [128, G + NACC], f32, kind="ExternalOutput")
    c2_part = nc.dram_tensor("c2_part", [F, F], f32, kind="ExternalOutput")

    with tile.TileContext(nc) as tc:
        with (
            tc.tile_pool(name="apool", bufs=10) as apool,
            tc.tile_pool(name="xpool", bufs=2) as xpool,
            tc.tile_pool(name="ypool", bufs=2) as ypool,
            tc.tile_pool(name="dpool", bufs=2) as dpool,
            tc.tile_pool(name="qpool", bufs=2) as qpool,
            tc.tile_pool(name="opool", bufs=1) as opool,
            tc.tile_pool(name="psum", bufs=1, space=bass.MemorySpace.PSUM) as psum,
        ):
            cps1 = psum.tile([128, G], f32, tag="cps1")
            cps2 = psum.tile([F, F], f32, tag="cps2")
            acc = opool.tile([128, NACC], f32)

            # per-partition contiguous views
            xv = xs[:].rearrange("(p t) d -> p (t d)", p=128)
            yv = ys[:].rearrange("(p t) d -> p (t d)", p=128)

            n_t = NCHK * TPC // 2   # DoubleRow: one matmul per tile pair
            ti = 0
            si = 0
            rings = (nc.sync, nc.scalar)
            pend_sq = []  # (piece, dtile) awaiting Square emission
            awc = aw[:].rearrange("(n p t) g -> n p (t g)", p=128, t=TPC)
            DR = mybir.MatmulPerfMode.DoubleRow
            PPC = TPC // 2  # DoubleRow tile pairs per chunk

            def mm_group(at3, cps, lo, hi, pairs):
                nonlocal ti, si
                if lo == 0:  # cps1: stationary cols 0:128, moving 0:192
                    for u in pairs:
                        nc.tensor.matmul(
                            cps[:], at3[:, 2 * u:2 * u + 2, 0:128],
                            at3[:, 2 * u:2 * u + 2, :], perf_mode=DR,
                            start=(ti == 0), stop=(ti == n_t - 1),
                        )
                        ti += 1
                else:        # cps2: stationary = moving = cols 128:192
                    for u in pairs:
                        rhs2 = at3[:, 2 * u:2 * u + 2, 128:G]
                        nc.tensor.matmul(
                            cps[:], rhs2, rhs2, perf_mode=DR,
                            start=(si == 0), stop=(si == n_t - 1),
                        )
                        si += 1

            for c in range(NCHK - 1):
                at = apool.tile([128, TPC * G], fp8)
                rings[c % 2].dma_start(at[:], awc[c])
                at3 = at[:].rearrange("p (t g) -> p t g", t=TPC)
                mm_group(at3, cps1, 0, 128, range(PPC))
                mm_group(at3, cps2, 128, G, range(PPC))

                # piece p: x at chunk 3p+3, y at 3p+4, opposite ring --
                # offset past the first chunks so the DMA ramp stays clean
                oring = rings[(c + 1) % 2]
                if c >= 3 and c % 3 == 0 and c < 3 * NCH + 3:
                    p = c // 3 - 1
                    xt = xpool.tile([128, CHW], fp8)
                    oring.dma_start(xt[:], xv[:, p * CHW:(p + 1) * CHW])
                elif c >= 4 and c % 3 == 1 and c < 3 * NCH + 4:
                    p = (c - 1) // 3 - 1
                    yt = ypool.tile([128, CHW], fp8)
                    oring.dma_start(yt[:], yv[:, p * CHW:(p + 1) * CHW])
                    dt_ = dpool.tile([128, CHW], bf16)
                    nc.vector.tensor_sub(dt_[:, :CHH], xt[:, :CHH], yt[:, :CHH])
                    nc.vector.tensor_sub(dt_[:, CHH:], xt[:, CHH:], yt[:, CHH:])
                    pend_sq.append((c + 2, 2 * p, dt_[:, :CHH]))
                    pend_sq.append((c + 3, 2 * p + 1, dt_[:, CHH:]))
                while pend_sq and c >= pend_sq[0][0]:
                    _, col, dh = pend_sq.pop(0)
                    qt = qpool.tile([128, CHH], f32)
                    nc.scalar.activation(
                        qt[:], dh,
                        mybir.ActivationFunctionType.Square,
                        accum_out=acc[:, col:col + 1],
                    )

            # final chunk as two 1024-row halves: halves the post-stream
            # matmul drain.  h0 on scalar (chunk 14 used sync), h1 on sync.
            aw2 = aw[:].rearrange("(n p t) g -> n p (t g)", p=128, t=TPC // 2)
            HP = PPC // 2  # pairs per half
            ath0 = apool.tile([128, TPC // 2 * G], fp8, tag="ath0", bufs=1)
            nc.scalar.dma_start(ath0[:], aw2[2 * NCHK - 2])
            # remaining Square halves while the tail halves stream
            while pend_sq:
                _, col, dh = pend_sq.pop(0)
                qt = qpool.tile([128, CHH], f32)
                nc.scalar.activation(
                    qt[:], dh,
                    mybir.ActivationFunctionType.Square,
                    accum_out=acc[:, col:col + 1],
                )
            a30 = ath0[:].rearrange("p (t g) -> p t g", t=TPC // 2)
            ath1 = apool.tile([128, TPC // 2 * G], fp8, tag="ath1", bufs=1)
            nc.sync.dma_start(ath1[:], aw2[2 * NCHK - 1])
            a31 = ath1[:].rearrange("p (t g) -> p t g", t=TPC // 2)
            mm_group(a30, cps1, 0, 128, range(HP))
            mm_group(a30, cps2, 128, G, range(HP))
            mm_group(a31, cps2, 128, G, range(HP))   # si hits stop here
            csb2 = opool.tile([F, F], f32, tag="csb2")
            nc.vector.tensor_copy(csb2[:], cps2[:])
            nc.scalar.dma_start(c2_part[:], csb2[:])
            mm_group(a31, cps1, 0, 128, range(HP))   # ti hits stop here

            # epilogue: PSUM -> SBUF -> DRAM (gram extraction happens on
            # host; csb2 was already written back above).
            # The SSE partials ride along as NCH extra columns of c1_part.
            csb1 = opool.tile([128, G + NACC], f32, tag="csb1")
            nc.vector.tensor_copy(csb1[:, G:], acc[:])
            nc.vector.tensor_copy(csb1[:, :G], cps1[:])
            nc.sync.dma_start(c1_part[:], csb1[:])

    nc.finalize()
    return nc


def _get_prog() -> bass.Bass:
    global _prog
    if _prog is None:
        _prog = _build()
    return _prog


def _in_maps(x_batch, y_batch, conv_w):
    A8 = (conv_w.reshape(M, G) * np.float32(ASCALE)).astype(ml_dtypes.float8_e4m3)
    x16 = (x_batch * np.float32(XSCALE)).astype(ml_dtypes.float8_e4m3)
    y16 = (y_batch * np.float32(XSCALE)).astype(ml_dtypes.float8_e4m3)
    maps = []
    for c in range(N_CORES):
        maps.append({
            "xs": np.ascontiguousarray(x16[c * ROWS:(c + 1) * ROWS]),
            "ys": np.ascontiguousarray(y16[c * ROWS:(c + 1) * ROWS]),
            "aw": np.ascontiguousarray(A8[c * MC:(c + 1) * MC]),
        })
    return maps


def _epilogue(C: np.ndarray, sse: float) -> np.ndarray:
    # C carries the fp8 pre-scale squared; it cancels in sim
    gram = C[0::KW, 0::KW] + C[1::KW, 1::KW] + C[2::KW, 2::KW]
    norms = np.sqrt(np.diag(gram))
    sim = gram / np.outer(norms, norms)
    mask = (sim > TAU) & (sim <= 1.0) & (~np.eye(F, dtype=bool))
    reg = sim[mask].sum()
    loss = sse / float(B * D) + ALPHA * reg
    return np.asarray(np.float32(loss))


def kernel(x_batch: np.ndarray, y_batch: np.ndarray, conv_w: np.ndarray) -> np.ndarray:
    nc = _get_prog()
    res = run_bass_kernel_spmd(
        nc, _in_maps(x_batch, y_batch, conv_w), core_ids=list(range(N_CORES))
    ).results
    C = np.zeros((G, G), np.float64)
    sse = 0.0
    for r in res:
        C[:128] += r["c1_part"][:, :G].astype(np.float64)
        C[128:, 128:] += r["c2_part"].astype(np.float64)
        sse += float(r["c1_part"][:, G:].sum(dtype=np.float64))
    sse /= float(XSCALE) ** 2
    # C is symmetric: mirror the block the cores didn't compute
    C[128:, :128] = C[:128, 128:].T
    return _epilogue(C, sse)
